# revision 7
# baseline (speedup 1.0000x reference)
"""ALiBi causal attention on 8 TRN2 NeuronCores.

Sharding: batch (4) x head-group (2 groups of 8 heads) = 8 cores.
Per core: QKV projection for its 8 heads, banded causal attention
(ALiBi decay makes k < q-127 contribute exactly 0 in fp32, so only a
256-wide k-band per query is computed), pairwise AllGather of the
attention output, then each core of the pair computes half the output
rows of the out-projection. Matmuls run in float32r (full fp32 storage,
single-pass PE mode, ~1.5e-4 rel err).

Self-contained: only needs numpy/jax/concourse (available on the
grading container via the axon site packages).
"""
import numpy as np


def _bf16_dtype():
    import ml_dtypes

    return np.dtype(ml_dtypes.bfloat16)


B, S, D = 4, 2048, 1024
H, HD = 16, 64
NCORES = 8
HEADS_PER_CORE = 8
FLOC = HEADS_PER_CORE * HD  # 512 local features
NEG = np.float32(-1e30)

_CACHE = {}


def _build(no_collective=False, bf16=True):
    import concourse.mybir as mybir
    import concourse.tile as tile
    from concourse import bacc

    F32 = mybir.dt.float32
    F32R = mybir.dt.float32r
    BF16 = mybir.dt.bfloat16
    PVDT = BF16 if bf16 else F32R
    AF = mybir.ActivationFunctionType
    ADD = mybir.AluOpType.add
    MULT = mybir.AluOpType.mult

    nc = bacc.Bacc("TRN2", target_bir_lowering=False, debug=False, num_devices=NCORES)

    xT = nc.dram_tensor("xT", [D, S], F32, kind="ExternalInput").ap()
    wqkvT = nc.dram_tensor("wqkvT", [D, 3 * FLOC], F32, kind="ExternalInput").ap()
    woT = nc.dram_tensor("woT", [D, FLOC], PVDT, kind="ExternalInput").ap()
    maskc = nc.dram_tensor("maskc", [128, 256], F32, kind="ExternalInput").ap()
    onesc = nc.dram_tensor("onesc", [128, 128], F32, kind="ExternalInput").ap()
    bqk = nc.dram_tensor("bqk", [128, 8], F32, kind="ExternalInput").ap()
    bvrow = nc.dram_tensor("bvrow", [1, FLOC], F32, kind="ExternalInput").ap()
    bo = nc.dram_tensor("bo", [128, 4], F32, kind="ExternalInput").ap()
    out = nc.dram_tensor("out", [FLOC, S], F32, kind="ExternalOutput").ap()

    xT3 = xT.rearrange("(kt p) s -> p kt s", p=128)       # [128, 8, 2048]
    w3 = wqkvT.rearrange("(kt p) f -> p kt f", p=128)     # [128, 8, 1536]
    wo3 = woT.rearrange("(kt p) f -> p kt f", p=128)      # [128, 8, 512]

    NKT = S // 128   # 16 k-tiles
    NQT = S // 256   # 8 q-tiles

    from contextlib import ExitStack
    with tile.TileContext(nc) as tc:
        with (
            tc.tile_pool(name="const", bufs=1) as cpool,
            tc.tile_pool(name="dram", bufs=1, space="DRAM") as dram,
            ExitStack() as outer,
        ):
            stageAB = outer.enter_context(ExitStack())
            qkvpool = stageAB.enter_context(tc.tile_pool(name="qkv", bufs=1))
            stageA = ExitStack()
            wpool = stageA.enter_context(tc.tile_pool(name="w", bufs=1))
            xpool = stageA.enter_context(tc.tile_pool(name="xin", bufs=2))
            psA = stageA.enter_context(tc.tile_pool(name="psA", bufs=4, space="PSUM"))
            mask_sb = cpool.tile([128, 256], F32)
            ones_sb = cpool.tile([128, 128], F32R)
            bqk_sb = cpool.tile([128, 8], F32)
            bv_sb = cpool.tile([1, FLOC], F32R)
            bo_sb = cpool.tile([128, 4], F32)
            nc.sync.dma_start(mask_sb[:], maskc)
            nc.sync.dma_start(ones_sb[:], onesc.bitcast(F32R))
            nc.sync.dma_start(bqk_sb[:], bqk)
            nc.sync.dma_start(bv_sb[:], bvrow.bitcast(F32R))
            nc.sync.dma_start(bo_sb[:], bo)

            ones_pv = cpool.tile([128, 64], PVDT)
            nc.vector.memset(ones_pv[:], 1.0)
            w_sb = wpool.tile([128, 8, 3 * FLOC], F32R)
            nc.sync.dma_start(w_sb[:], w3.bitcast(F32R))

            qt_sb = qkvpool.tile([128, 4, S], F32R, tag="q")
            kt_sb = qkvpool.tile([128, 4, S], F32R, tag="k")
            v_sb = qkvpool.tile([128, NKT, FLOC], PVDT, tag="v")

            # ---- Stage A: projections, per s-chunk of 512 ----
            for sc in range(4):
                s0 = sc * 512
                xt = xpool.tile([128, 8, 512], F32R, tag="xt")
                nc.sync.dma_start(xt[:], xT3[:, :, s0 : s0 + 512].bitcast(F32R))
                # Q (mi 0..3) and K (mi 4..7): features on partitions
                for mi in range(8):
                    ps = psA.tile([128, 512], F32, tag="a")
                    for kt in range(8):
                        nc.tensor.matmul(
                            ps[:],
                            w_sb[:, kt, mi * 128 : mi * 128 + 128],
                            xt[:, kt, :],
                            start=(kt == 0),
                            stop=(kt == 7),
                        )
                    dst = qt_sb if mi < 4 else kt_sb
                    nc.scalar.activation(
                        dst[:, mi % 4, s0 : s0 + 512],
                        ps[:],
                        AF.Identity,
                        bias=bqk_sb[:, mi : mi + 1],
                    )
                # V (natural layout): s on partitions
                for si in range(4):
                    ps = psA.tile([128, 512], F32, tag="a")
                    nc.tensor.matmul(
                        ps[:], ones_sb[0:1, :], bv_sb[:], start=True, stop=False
                    )
                    for kt in range(8):
                        nc.tensor.matmul(
                            ps[:],
                            xt[:, kt, si * 128 : si * 128 + 128],
                            w_sb[:, kt, 2 * FLOC : 3 * FLOC],
                            start=False,
                            stop=(kt == 7),
                        )
                    nc.any.tensor_copy(v_sb[:, sc * 4 + si, :], ps[:])

            stageA.close()

            # ---- Stage B: attention per head ----
            # normalized attention pieces go straight to DRAM (ag_in)
            ag_in = dram.tile([FLOC, S], PVDT)
            spool = stageAB.enter_context(tc.tile_pool(name="small", bufs=4))
            ptpool = stageAB.enter_context(tc.tile_pool(name="pt", bufs=3))
            psS = stageAB.enter_context(tc.tile_pool(name="psS", bufs=2, space="PSUM"))
            psV = stageAB.enter_context(tc.tile_pool(name="psV", bufs=4, space="PSUM"))
            for h in range(HEADS_PER_CORE):
                mi_h, po = h // 2, (h % 2) * 64
                KTh = kt_sb[po : po + 64, mi_h, :]
                QTh = qt_sb[po : po + 64, mi_h, :]
                Vh = lambda kt: v_sb[:, kt, h * HD : (h + 1) * HD]

                # scores + exp per k-group of 4 k-tiles
                pts = []
                for G in range(4):
                    scps = psS.tile([128, 1024], F32, tag="sc")
                    pt = ptpool.tile([128, 4, 256], PVDT, tag="pt")
                    ngrp = 4 if G < 3 else 3  # tile 15 is 128 wide
                    for j in range(4):
                        kt = 4 * G + j
                        k0 = kt * 128
                        qn = min(256, S - k0)
                        nc.tensor.matmul(
                            scps[:, j * 256 : j * 256 + qn],
                            KTh[:, k0 : k0 + 128],
                            QTh[:, k0 : k0 + qn],
                            start=True,
                            stop=True,
                        )
                    wid = ngrp * 256
                    sc3 = scps[:, 0:wid].rearrange("p (g f) -> p g f", f=256)
                    nc.vector.tensor_tensor(
                        sc3, sc3, mask_sb[:, None, :].to_broadcast((128, ngrp, 256)), ADD
                    )
                    nc.scalar.activation(
                        pt[:, 0:ngrp, :].rearrange("p g f -> p (g f)"),
                        scps[:, 0:wid],
                        AF.Exp,
                    )
                    if G == 3:
                        # last k-tile: 128 valid columns
                        nc.vector.tensor_tensor(
                            scps[:, 768:896], scps[:, 768:896],
                            mask_sb[:, 0:128], ADD,
                        )
                        nc.scalar.activation(pt[:, 3, 0:128], scps[:, 768:896], AF.Exp)
                    pts.append(pt)

                # PV + denominators per q-tile of 256
                for qt in range(NQT):
                    q0 = qt * 256
                    pvden = psV.tile([128, 256], F32, tag="pvden")
                    pv = pvden[0:64, :]
                    den = pvden[64:128, :]
                    ktB = 2 * qt      # full [0:256]
                    ktA = 2 * qt - 1  # cols 128:256 -> pv[0:128]
                    ktC = 2 * qt + 1  # cols 0:128 -> pv[128:256]
                    rhsB = pts[ktB // 4][:, ktB % 4, :]
                    nc.tensor.matmul(pv[:], Vh(ktB), rhsB, start=True, stop=False)
                    nc.tensor.matmul(den[:], ones_pv[:], rhsB, start=True, stop=False)
                    if ktA >= 0:
                        rhsA = pts[ktA // 4][:, ktA % 4, 128:256]
                        nc.tensor.matmul(pv[:, 0:128], Vh(ktA), rhsA, start=False, stop=False)
                        nc.tensor.matmul(den[:, 0:128], ones_pv[:], rhsA, start=False, stop=False)
                    if ktC < NKT:
                        rhsC = pts[ktC // 4][:, ktC % 4, 0:128]
                        nc.tensor.matmul(pv[:, 128:256], Vh(ktC), rhsC, start=False, stop=True)
                        nc.tensor.matmul(den[:, 128:256], ones_pv[:], rhsC, start=False, stop=True)
                    rec = spool.tile([64, 256], F32R, tag="rec")
                    with nc.allow_low_precision(reason="f32r rounding only"):
                        nc.vector.reciprocal(rec[:], den[:])
                    anorm = spool.tile([64, 256], PVDT, tag="anorm")
                    nc.vector.tensor_tensor(anorm[:], pv[:], rec[:], MULT)
                    r0 = mi_h * 128 + po
                    nc.sync.dma_start(ag_in[r0 : r0 + 64, q0 : q0 + 256], anorm[:])

            stageAB.close()

            # ---- Stage C: pairwise AllGather + half out-projection ----
            stageC = outer.enter_context(ExitStack())
            opool = stageC.enter_context(tc.tile_pool(name="oproj", bufs=2))
            wopool = stageC.enter_context(tc.tile_pool(name="wo", bufs=1))
            ytpool = stageC.enter_context(tc.tile_pool(name="yt", bufs=3))
            psC = stageC.enter_context(tc.tile_pool(name="psC", bufs=2, space="PSUM"))
            ag_out = dram.tile([2 * FLOC, S], PVDT)
            if no_collective:
                nc.sync.dma_start(ag_out[0:FLOC, :], ag_in[:])
                nc.sync.dma_start(ag_out[FLOC : 2 * FLOC, :], ag_in[:])
            else:
                nc.gpsimd.collective_compute(
                    "AllGather",
                    mybir.AluOpType.bypass,
                    replica_groups=[[0, 1], [2, 3], [4, 5], [6, 7]],
                    ins=[ag_in[:].opt()],
                    outs=[ag_out[:].opt()],
                )
            ag3 = ag_out[:].rearrange("(kt p) s -> p kt s", p=128)  # [128, 8, 2048]
            wo_sb = wopool.tile([128, 8, FLOC], PVDT, tag="wo")
            nc.sync.dma_start(wo_sb[:], wo3)
            for sb in range(4):
                s0 = sb * 512
                at = opool.tile([128, 8, 512], PVDT, tag="at")
                nc.sync.dma_start(at[:], ag3[:, :, s0 : s0 + 512])
                for mi in range(4):
                    ps = psC.tile([128, 512], F32, tag="c")
                    for kt in range(8):
                        nc.tensor.matmul(
                            ps[:],
                            wo_sb[:, kt, mi * 128 : mi * 128 + 128],
                            at[:, kt, :],
                            start=(kt == 0),
                            stop=(kt == 7),
                        )
                    yt = ytpool.tile([128, 512], F32, tag="yt")
                    nc.scalar.activation(
                        yt[:], ps[:], AF.Identity, bias=bo_sb[:, mi : mi + 1]
                    )
                    nc.sync.dma_start(
                        out[mi * 128 : mi * 128 + 128, s0 : s0 + 512], yt[:]
                    )
    nc.compile()
    return nc


def _get_runner():
    if "runner" in _CACHE:
        return _CACHE["runner"]
    import jax
    import numpy as _np
    from jax.sharding import Mesh, PartitionSpec, NamedSharding
    from jax.experimental.shard_map import shard_map
    import concourse.mybir as mybir
    from concourse.bass2jax import (
        _bass_exec_p,
        install_neuronx_cc_hook,
        partition_id_tensor,
    )

    nc = _build()
    install_neuronx_cc_hook()
    partition_name = nc.partition_id_tensor.name if nc.partition_id_tensor else None

    in_names, out_names, out_avals, zero_outs = [], [], [], []
    for alloc in nc.m.functions[0].allocations:
        if not isinstance(alloc, mybir.MemoryLocationSet):
            continue
        name = alloc.memorylocations[0].name
        if alloc.kind == "ExternalInput":
            if name != partition_name:
                in_names.append(name)
        elif alloc.kind == "ExternalOutput":
            shape = tuple(alloc.tensor_shape)
            dtype = mybir.dt.np(alloc.dtype)
            out_names.append(name)
            out_avals.append(jax.core.ShapedArray(shape, dtype))
            zero_outs.append(_np.zeros(shape, dtype))

    n_params = len(in_names)
    all_in_names = list(in_names) + list(out_names)
    if partition_name is not None:
        all_in_names.append(partition_name)

    def _body(*args):
        operands = list(args)
        if partition_name is not None:
            operands.append(partition_id_tensor())
        outs = _bass_exec_p.bind(
            *operands,
            out_avals=tuple(out_avals),
            in_names=tuple(all_in_names),
            out_names=tuple(out_names),
            lowering_input_output_aliases=(),
            sim_require_finite=True,
            sim_require_nnan=True,
            nc=nc,
        )
        return tuple(outs)

    devices = jax.devices()[:NCORES]
    mesh = Mesh(np.asarray(devices), ("core",))
    in_specs = (PartitionSpec("core"),) * (n_params + len(out_names))
    out_specs = (PartitionSpec("core"),) * len(out_names)
    fn = jax.jit(
        shard_map(_body, mesh=mesh, in_specs=in_specs, out_specs=out_specs,
                  check_rep=False),
        keep_unused=True,
    )
    sharding = NamedSharding(mesh, PartitionSpec("core"))
    runner = {
        "fn": fn,
        "in_names": in_names,
        "out_names": out_names,
        "out_avals": out_avals,
        "zero_outs": zero_outs,
        "sharding": sharding,
    }
    _CACHE["runner"] = runner
    return runner


def _prep_inputs(x, w_qkv, b_qkv, w_out, b_out):
    """Shard + lay out host-side. Returns list of per-core dicts."""
    x = np.asarray(x, np.float32)
    w_qkv = np.asarray(w_qkv, np.float32)
    b_qkv = np.asarray(b_qkv, np.float32)
    w_out = np.asarray(w_out, np.float32)
    b_out = np.asarray(b_out, np.float32)

    p_ = np.arange(128)[:, None]
    f_ = np.arange(256)[None, :]
    maskc = np.where(f_ >= p_, (p_ - f_).astype(np.float32), NEG)
    onesc = np.ones((128, 128), np.float32)

    scale = np.float32(1.0 / np.sqrt(HD))
    in_maps = []
    for c in range(NCORES):
        b, g = c // 2, c % 2
        fsl = slice(g * FLOC, (g + 1) * FLOC)
        wq = w_qkv[0 * D :][fsl, :] * scale
        wk = w_qkv[1 * D : 2 * D][fsl, :]
        wv = w_qkv[2 * D : 3 * D][fsl, :]
        bq = b_qkv[0 * D :][fsl] * scale
        bk = b_qkv[1 * D : 2 * D][fsl]
        bv = b_qkv[2 * D : 3 * D][fsl]
        osl = slice((c % 2) * FLOC, (c % 2 + 1) * FLOC)
        in_maps.append(
            {
                "xT": np.ascontiguousarray(x[b].T),
                "wqkvT": np.ascontiguousarray(
                    np.concatenate([wq, wk, wv], axis=0).T
                ),
                "woT": np.ascontiguousarray(w_out[osl, :].T).astype(_bf16_dtype()),
                "maskc": maskc,
                "onesc": onesc,
                "bqk": np.ascontiguousarray(
                    np.concatenate([bq, bk]).reshape(8, 128).T
                ),
                "bvrow": bv.reshape(1, FLOC),
                "bo": np.ascontiguousarray(b_out[osl].reshape(4, 128).T),
            }
        )
    return in_maps


def _run_device(in_maps):
    import jax

    r = _get_runner()
    n = NCORES
    concat_in = [
        np.concatenate([np.asarray(in_maps[c][name]) for c in range(n)], axis=0)
        for name in r["in_names"]
    ]
    concat_zero = [
        np.zeros((n * z.shape[0], *z.shape[1:]), z.dtype) for z in r["zero_outs"]
    ]
    args = [jax.device_put(a, r["sharding"]) for a in concat_in + concat_zero]
    outs = r["fn"](*args)
    jax.block_until_ready(outs)
    oname = r["out_names"].index("out")
    full = np.asarray(outs[oname]).reshape(n, FLOC, S)
    return full, args


def kernel(x, w_qkv, b_qkv, w_out, b_out):
    in_maps = _prep_inputs(x, w_qkv, b_qkv, w_out, b_out)
    full, _ = _run_device(in_maps)
    # core 2b has y^T rows 0:512, core 2b+1 rows 512:1024 for batch b
    y = np.empty((B, S, D), np.float32)
    for b in range(B):
        yt = np.concatenate([full[2 * b], full[2 * b + 1]], axis=0)  # [1024, 2048]
        y[b] = yt.T
    return y


# revision 30
# speedup vs baseline: 1.2737x; 1.2737x over previous
"""ALiBi causal attention on 8 TRN2 NeuronCores — no-communication variant.

Sharding: batch (4) x query-half (2) = 8 cores, zero collectives.
Each core receives a HOST-WINDOWED input xT covering key positions
[Q0-128, Q0+1024) of its batch (front-padded with zeros on even cores)
plus its query half xqT. It computes K/V for the 9-k-tile window,
Q for its half, banded causal attention (ALiBi decay makes k < q-127
contribute exactly 0 in fp32), and the full out-projection for its
query half. The window edge tile's mask is a per-core input (zeros on
even cores to kill the padding). All matmuls in bf16.
"""
import numpy as np


def _bf16_dtype():
    import ml_dtypes

    return np.dtype(ml_dtypes.bfloat16)


B, S, D = 4, 2048, 1024
H, HD = 16, 64
NCORES = 8
QH = S // 2          # 1024 queries per core
KW = QH + 128        # 1152 key-window positions per core (9 k-tiles)
NKT = KW // 128      # 9 local k-tiles
NQT = QH // 256      # 4 local q-tiles
NEG = np.float32(-1e30)

_CACHE = {}


def _build():
    import concourse.mybir as mybir
    import concourse.tile as tile
    from concourse import bacc
    from contextlib import ExitStack

    F32 = mybir.dt.float32
    BF16 = mybir.dt.bfloat16
    AF = mybir.ActivationFunctionType
    MULT = mybir.AluOpType.mult

    nc = bacc.Bacc("TRN2", target_bir_lowering=False, debug=False, num_devices=NCORES)

    xT = nc.dram_tensor("xT", [D, KW], BF16, kind="ExternalInput").ap()
    xqT = nc.dram_tensor("xqT", [D, QH], BF16, kind="ExternalInput").ap()
    wqkvT = nc.dram_tensor("wqkvT", [D, 3 * D], BF16, kind="ExternalInput").ap()
    woT = nc.dram_tensor("woT", [D, D], BF16, kind="ExternalInput").ap()
    m2c = nc.dram_tensor("m2c", [128, 256], BF16, kind="ExternalInput").ap()
    m2e = nc.dram_tensor("m2e", [128, 128], BF16, kind="ExternalInput").ap()
    onesc = nc.dram_tensor("onesc", [128, 128], BF16, kind="ExternalInput").ap()
    bqk = nc.dram_tensor("bqk", [128, 16], F32, kind="ExternalInput").ap()
    bvrow = nc.dram_tensor("bvrow", [1, D], BF16, kind="ExternalInput").ap()
    bo = nc.dram_tensor("bo", [128, 8], F32, kind="ExternalInput").ap()
    out = nc.dram_tensor("out", [D, QH], F32, kind="ExternalOutput").ap()

    xT3 = xT.rearrange("(kt p) s -> p kt s", p=128)     # [128, 8, 1152]
    xq3 = xqT.rearrange("(kt p) s -> p kt s", p=128)    # [128, 8, 1024]
    w3 = wqkvT.rearrange("(kt p) f -> p kt f", p=128)   # [128, 8, 3072]
    wo3 = woT.rearrange("(kt p) f -> p kt f", p=128)    # [128, 8, 1024]

    with tile.TileContext(nc) as tc:
        with (
            tc.tile_pool(name="const", bufs=1) as cpool,
            tc.tile_pool(name="dram", bufs=1, space="DRAM") as dram,
            tc.tile_pool(name="qkv", bufs=1) as qkvpool,
            ExitStack() as outer,
        ):
            m2_sb = cpool.tile([128, 256], BF16)
            m2e_sb = cpool.tile([128, 128], BF16)
            ones_sb = cpool.tile([128, 128], BF16)
            bqk_sb = cpool.tile([128, 16], F32)
            bv_sb = cpool.tile([1, D], BF16)
            bo_sb = cpool.tile([128, 8], F32)
            nc.sync.dma_start(m2_sb[:], m2c)
            nc.sync.dma_start(m2e_sb[:], m2e)
            nc.sync.dma_start(ones_sb[:], onesc)
            nc.sync.dma_start(bqk_sb[:], bqk)
            nc.sync.dma_start(bv_sb[:], bvrow)
            nc.sync.dma_start(bo_sb[:], bo)
            ones_pv = cpool.tile([128, 64], BF16)
            nc.vector.memset(ones_pv[:], 1.0)

            # K^T tiles: [128, 8 m, 128] per local k-tile (9); V: per k-tile [128, 1024]
            ktile_sb = [
                qkvpool.tile([128, 8, 128], BF16, tag=f"kk{k}", name=f"kk{k}")
                for k in range(NKT)
            ]
            vtile_sb = [
                qkvpool.tile([128, H, 2 * HD], BF16, tag=f"vv{k}", name=f"vv{k}")
                for k in range(NKT)
            ]
            for k in range(NKT):
                nc.gpsimd.memset(vtile_sb[k][:, :, HD : 2 * HD], 1.0)
            qt_cs = [
                qkvpool.tile([128, 8, 512], BF16, tag=f"q{sc}", name=f"qt{sc}")
                for sc in range(2)
            ]
            attn_d = dram.tile([D, QH], BF16)

            opool = outer.enter_context(tc.tile_pool(name="oproj", bufs=2))
            wopool = outer.enter_context(tc.tile_pool(name="wo", bufs=1))
            ytpool = outer.enter_context(tc.tile_pool(name="yt", bufs=3))
            psA = outer.enter_context(tc.tile_pool(name="psA", bufs=2, space="PSUM"))
            stageA = outer.enter_context(ExitStack())
            wpool = stageA.enter_context(tc.tile_pool(name="w", bufs=1))
            xpool = stageA.enter_context(tc.tile_pool(name="xin", bufs=2))

            w_sb = wpool.tile([128, 8, 3 * D], BF16)
            for kt in range(8):
                for j in range(2):
                    nc.sync.dma_start(
                        w_sb[:, kt, j * 1536 : (j + 1) * 1536],
                        w3[:, kt, j * 1536 : (j + 1) * 1536],
                    )

            # ---- Stage A: K/V over the 9-tile window (chunks of 384=3 k-tiles),
            #      Q over the local half (2 chunks of 512) ----
            for vc in range(3):
                s0 = vc * 384
                xt = xpool.tile([128, 8, 384], BF16, tag="xt", name=f"xtv{vc}")
                for kt in range(8):
                    nc.sync.dma_start(xt[:, kt, :], xT3[:, kt, s0 : s0 + 384])
                # K: m-tiles 0..7, N=384
                for mi in range(8):
                    ps = psA.tile([128, 384], F32, tag="a", name=f"kp{vc}_{mi}")
                    for kt in range(8):
                        nc.tensor.matmul(
                            ps[:],
                            w_sb[:, kt, D + mi * 128 : D + mi * 128 + 128],
                            xt[:, kt, :],
                            start=(kt == 0),
                            stop=(kt == 7),
                        )
                    for j in range(3):
                        nc.scalar.activation(
                            ktile_sb[vc * 3 + j][:, mi, :],
                            ps[:, j * 128 : j * 128 + 128],
                            AF.Identity,
                            bias=bqk_sb[:, 8 + mi : 8 + mi + 1],
                        )
                # V: 3 s-subtiles of 128, f=1024 in 2 halves
                for si in range(3):
                    for fh in range(2):
                        ps = psA.tile([128, 512], F32, tag="a", name=f"vp{vc}_{si}_{fh}")
                        nc.tensor.matmul(
                            ps[:],
                            ones_sb[0:1, :],
                            bv_sb[:, fh * 512 : fh * 512 + 512],
                            start=True,
                            stop=False,
                        )
                        for kt in range(8):
                            nc.tensor.matmul(
                                ps[:],
                                xt[:, kt, si * 128 : si * 128 + 128],
                                w_sb[:, kt, 2 * D + fh * 512 : 2 * D + fh * 512 + 512],
                                start=False,
                                stop=(kt == 7),
                            )
                        nc.scalar.activation(
                            vtile_sb[vc * 3 + si][:, fh * 8 : fh * 8 + 8, 0:HD],
                            ps[:].rearrange("p (h d) -> p h d", d=HD),
                            AF.Identity,
                            bias=0.0,
                        )
            for qc in range(2):
                s0 = qc * 512
                xt = xpool.tile([128, 8, 512], BF16, tag="xtq", name=f"xtq{qc}")
                for kt in range(8):
                    nc.sync.dma_start(xt[:, kt, :], xq3[:, kt, s0 : s0 + 512])
                for mi in range(8):
                    ps = psA.tile([128, 512], F32, tag="a", name=f"qp{qc}_{mi}")
                    for kt in range(8):
                        nc.tensor.matmul(
                            ps[:],
                            w_sb[:, kt, mi * 128 : mi * 128 + 128],
                            xt[:, kt, :],
                            start=(kt == 0),
                            stop=(kt == 7),
                        )
                    nc.scalar.activation(
                        qt_cs[qc][:, mi, :],
                        ps[:],
                        AF.Identity,
                        bias=bqk_sb[:, mi : mi + 1],
                    )
            # ---- Stage B: attention, 16 heads, local q in [0, 1024) ----
            # local k-tile K covers keys [128K, 128K+128); valid/banded q-range
            # of K: [max(0, 128K-128), min(128K+127+... , 1024)) -> width 128
            # for K=0 and K=8, else 256, starting at qstart(K) = max(0, 128K-128).
            wo_sb = wopool.tile([128, 8, D], BF16, tag="wo")
            for kt in range(8):
                nc.sync.dma_start(wo_sb[:, kt, :], wo3[:, kt, :])
            a3 = attn_d[:].rearrange("(kt p) s -> p kt s", p=128)
            at_sb = [
                opool.tile([128, 8, 512], BF16, tag=f"at{sb}", name=f"at{sb}")
                for sb in range(2)
            ]

            stageA.close()
            stageB = outer.enter_context(ExitStack())
            spool = stageB.enter_context(tc.tile_pool(name="small", bufs=4))
            ptpool = stageB.enter_context(tc.tile_pool(name="pt", bufs=6))
            psS = stageB.enter_context(tc.tile_pool(name="psS", bufs=2, space="PSUM"))
            psV = stageB.enter_context(tc.tile_pool(name="psV", bufs=2, space="PSUM"))

            def qwin(K):
                qs = max(0, 128 * K - 128)
                qe = min(128 * K + 128, QH)
                return qs, qe - qs  # start, width

            def QTs(c0, w):
                return qt_cs[c0 // 512][
                    :, :, c0 % 512 : (c0 % 512) + w
                ]

            for h in range(H):
                mi_h, po = h // 2, (h % 2) * 64

                # scores+exp+mask per k-group: groups of slots (K 0-3, 4-7, 8)
                pts = []
                for G in range(3):
                    Ks = range(4 * G, min(4 * G + 4, NKT))
                    scps = psS.tile([128, 1024], F32, tag="sc", name=f"sc{h}_{G}")
                    pt = ptpool.tile([128, 4, 256], BF16, tag="pt", name=f"pt{h}_{G}")
                    for K in Ks:
                        j = K % 4
                        qs, w = qwin(K)
                        lhs = ktile_sb[K][po : po + 64, mi_h, :]
                        # q-window may cross the 512-chunk boundary of qt_cs
                        pieces = []
                        c = qs
                        while c < qs + w:
                            cw = min(512 - (c % 512), qs + w - c)
                            pieces.append((c, cw))
                            c += cw
                        off = 0
                        for (c0, cw) in pieces:
                            nc.tensor.matmul(
                                scps[:, j * 256 + off : j * 256 + off + cw],
                                lhs,
                                QTs(c0, cw)[po : po + 64, mi_h, :],
                                start=True,
                                stop=True,
                            )
                            off += cw
                    # exp + mask-mult grouped across the whole k-group
                    meng = nc.gpsimd if h % 2 == 0 else nc.vector
                    nG = len(list(Ks))
                    wid = (nG - 1) * 256 + qwin(max(Ks))[1]
                    if G == 0:
                        # slot 0 is the 128-wide edge tile; exp the full group
                        # (unread slot-0 tail included) then mask per region
                        nc.scalar.activation(
                            pt[:, 0:4, :].rearrange("p g f -> p (g f)"),
                            scps[:, 0:1024],
                            AF.Exp,
                        )
                        meng.tensor_tensor(
                            pt[:, 0, 0:128], pt[:, 0, 0:128], m2e_sb[:, 0:128], MULT
                        )
                        meng.tensor_tensor(
                            pt[:, 1:4, :],
                            pt[:, 1:4, :],
                            m2_sb[:, None, :].to_broadcast((128, 3, 256)),
                            MULT,
                        )
                    elif G == 1:
                        nc.scalar.activation(
                            pt[:, 0:4, :].rearrange("p g f -> p (g f)"),
                            scps[:, 0:1024],
                            AF.Exp,
                        )
                        meng.tensor_tensor(
                            pt[:, 0:4, :],
                            pt[:, 0:4, :],
                            m2_sb[:, None, :].to_broadcast((128, 4, 256)),
                            MULT,
                        )
                    else:  # G == 2: K8 only, 128 wide
                        nc.scalar.activation(pt[:, 0, 0:128], scps[:, 0:128], AF.Exp)
                        meng.tensor_tensor(
                            pt[:, 0, 0:128], pt[:, 0, 0:128], m2_sb[:, 0:128], MULT
                        )
                    pts.append(pt)

                # PV + replicated denominators in ONE matmul chain per q-tile:
                # lhsT = [V_h | ones] (M=128) -> rows 0:64 = pv, 64:128 = den.
                Vh = lambda K: vtile_sb[K][:, h, :]
                for qt in range(NQT):
                    q0 = qt * 256
                    pvden = psV.tile([128, 256], F32, tag="pvden", name=f"pv{h}_{qt}")
                    pv = pvden[0:64, :]
                    den = pvden[64:128, :]
                    KB = 2 * qt + 1   # covers [q0, q0+255] fully
                    KA = 2 * qt       # second 128 of its window -> cols 0:128
                    KC = 2 * qt + 2   # first 128 of its window -> cols 128:256
                    rhsB = pts[KB // 4][:, KB % 4, 0:256]
                    nc.tensor.matmul(pvden[:], Vh(KB), rhsB, start=True, stop=False)
                    qsA, _ = qwin(KA)
                    offA = q0 - qsA
                    rhsA = pts[KA // 4][:, KA % 4, offA : offA + 128]
                    nc.tensor.matmul(pvden[:, 0:128], Vh(KA), rhsA, start=False, stop=False)
                    qsC, _ = qwin(KC)
                    offC = q0 + 128 - qsC
                    rhsC = pts[KC // 4][:, KC % 4, offC : offC + 128]
                    nc.tensor.matmul(pvden[:, 128:256], Vh(KC), rhsC, start=False, stop=True)

                    rec = spool.tile([64, 256], F32, tag="rec", name=f"rc{h}_{qt}")
                    nc.vector.reciprocal(rec[:], den[:])
                    anorm = spool.tile([64, 256], BF16, tag="anorm", name=f"an{h}_{qt}")
                    nc.vector.tensor_tensor(anorm[:], pv[:], rec[:], MULT)
                    r0 = h * 64
                    nc.sync.dma_start(attn_d[r0 : r0 + 64, q0 : q0 + 256], anorm[:])
                if h % 2 == 1:
                    kt = h // 2
                    for sb in range(2):
                        nc.sync.dma_start(
                            at_sb[sb][:, kt, :], a3[:, kt, sb * 512 : sb * 512 + 512]
                        )
            # ---- Stage C: out-projection for the local q-half ----
            for sb in range(2):
                s0 = sb * 512
                at = at_sb[sb]
                for mi in range(8):
                    ps = psA.tile([128, 512], F32, tag="a", name=f"op{sb}_{mi}")
                    for kt in range(8):
                        nc.tensor.matmul(
                            ps[:],
                            wo_sb[:, kt, mi * 128 : mi * 128 + 128],
                            at[:, kt, :],
                            start=(kt == 0),
                            stop=(kt == 7),
                        )
                    yt = ytpool.tile([128, 512], F32, tag="yt", name=f"yt{sb}_{mi}")
                    nc.scalar.activation(
                        yt[:], ps[:], AF.Identity, bias=bo_sb[:, mi : mi + 1]
                    )
                    nc.sync.dma_start(
                        out[mi * 128 : mi * 128 + 128, s0 : s0 + 512], yt[:]
                    )
    nc.compile()
    return nc


def _prep_inputs(x, w_qkv, b_qkv, w_out, b_out):
    x = np.asarray(x, np.float32)
    w_qkv = np.asarray(w_qkv, np.float32)
    b_qkv = np.asarray(b_qkv, np.float32)
    w_out = np.asarray(w_out, np.float32)
    b_out = np.asarray(b_out, np.float32)
    bf16 = _bf16_dtype()

    p_ = np.arange(128)[:, None]
    f_ = np.arange(256)[None, :]
    with np.errstate(over="ignore", under="ignore"):
        m2c = np.where(f_ >= p_, np.exp((p_ - f_).astype(np.float64)), 0.0).astype(bf16)
    onesc = np.ones((128, 128), np.float32).astype(bf16)
    scale = np.float32(1.0 / np.sqrt(HD))

    wq = w_qkv[0:D] * scale
    wqkvT = np.ascontiguousarray(
        np.concatenate([wq, w_qkv[D : 2 * D], w_qkv[2 * D :]], axis=0).T
    ).astype(bf16)
    woT = np.ascontiguousarray(w_out.T).astype(bf16)
    bq = b_qkv[0:D] * scale
    bqk_h = np.ascontiguousarray(
        np.concatenate([bq, b_qkv[D : 2 * D]]).reshape(16, 128).T
    )
    bv = b_qkv[2 * D :].reshape(1, D).astype(bf16)
    bo_h = np.ascontiguousarray(b_out.reshape(8, 128).T)

    in_maps = []
    for c in range(NCORES):
        b, qh = c // 2, c % 2
        Q0 = qh * QH
        xw = np.zeros((KW, D), np.float32)
        lo = Q0 - 128
        src_lo = max(lo, 0)
        xw[src_lo - lo : KW] = x[b, src_lo : Q0 + QH]
        m2e = (
            m2c[:, 128:256]
            if qh == 1
            else np.zeros((128, 128), np.float32).astype(bf16)
        )
        in_maps.append(
            {
                "xT": np.ascontiguousarray(xw.T).astype(bf16),
                "xqT": np.ascontiguousarray(x[b, Q0 : Q0 + QH].T).astype(bf16),
                "wqkvT": wqkvT,
                "woT": woT,
                "m2c": m2c,
                "m2e": np.ascontiguousarray(m2e),
                "onesc": onesc,
                "bqk": bqk_h,
                "bvrow": bv,
                "bo": bo_h,
            }
        )
    return in_maps


def _get_runner():
    if "runner" in _CACHE:
        return _CACHE["runner"]
    import jax
    from jax.sharding import Mesh, PartitionSpec, NamedSharding
    from jax.experimental.shard_map import shard_map
    import concourse.mybir as mybir
    from concourse.bass2jax import (
        _bass_exec_p,
        install_neuronx_cc_hook,
        partition_id_tensor,
    )

    nc = _build()
    install_neuronx_cc_hook()
    partition_name = nc.partition_id_tensor.name if nc.partition_id_tensor else None
    in_names, out_names, out_avals, zero_outs = [], [], [], []
    for alloc in nc.m.functions[0].allocations:
        if not isinstance(alloc, mybir.MemoryLocationSet):
            continue
        name = alloc.memorylocations[0].name
        if alloc.kind == "ExternalInput":
            if name != partition_name:
                in_names.append(name)
        elif alloc.kind == "ExternalOutput":
            shape = tuple(alloc.tensor_shape)
            dtype = mybir.dt.np(alloc.dtype)
            out_names.append(name)
            out_avals.append(jax.core.ShapedArray(shape, dtype))
            zero_outs.append(np.zeros(shape, dtype))
    all_in = list(in_names) + list(out_names)
    if partition_name is not None:
        all_in.append(partition_name)

    def _body(*args):
        operands = list(args)
        if partition_name is not None:
            operands.append(partition_id_tensor())
        outs = _bass_exec_p.bind(
            *operands,
            out_avals=tuple(out_avals),
            in_names=tuple(all_in),
            out_names=tuple(out_names),
            lowering_input_output_aliases=(),
            sim_require_finite=True,
            sim_require_nnan=True,
            nc=nc,
        )
        return tuple(outs)

    devices = jax.devices()[:NCORES]
    mesh = Mesh(np.asarray(devices), ("core",))
    nio = len(in_names) + len(out_names)
    fn = jax.jit(
        shard_map(
            _body,
            mesh=mesh,
            in_specs=(PartitionSpec("core"),) * nio,
            out_specs=(PartitionSpec("core"),) * len(out_names),
            check_rep=False,
        ),
        keep_unused=True,
    )
    runner = {
        "fn": fn,
        "in_names": in_names,
        "out_names": out_names,
        "out_avals": out_avals,
        "zero_outs": zero_outs,
        "sharding": NamedSharding(mesh, PartitionSpec("core")),
    }
    _CACHE["runner"] = runner
    return runner


def kernel(x, w_qkv, b_qkv, w_out, b_out):
    import jax

    in_maps = _prep_inputs(x, w_qkv, b_qkv, w_out, b_out)
    r = _get_runner()
    n = NCORES
    concat_in = [
        np.concatenate([np.asarray(in_maps[c][name]) for c in range(n)], axis=0)
        for name in r["in_names"]
    ]
    concat_zero = [
        np.zeros((n * z.shape[0], *z.shape[1:]), z.dtype) for z in r["zero_outs"]
    ]
    args = [jax.device_put(a, r["sharding"]) for a in concat_in + concat_zero]
    outs = r["fn"](*args)
    jax.block_until_ready(outs)
    oname = r["out_names"].index("out")
    full = np.asarray(outs[oname]).reshape(n, D, QH)
    y = np.empty((B, S, D), np.float32)
    for b in range(B):
        yt = np.concatenate([full[2 * b], full[2 * b + 1]], axis=1)  # [1024, 2048]
        y[b] = yt.T
    return y
